# revision 18
# baseline (speedup 1.0000x reference)
"""LIF neuron (leaky integrate, bidirectional threshold fire, hard reset)
on 8 Trainium2 NeuronCores.

Math (per element, recurrence over T):
    v      = 0.4*V + x_t
    out_t  = (v >= 1) - (v <= -1)               in {-1, 0, +1}
    V'     = (1 - |out_t|) * v                  (hard reset to 0)

Device encoding trick: e = int8(RNE(0.5*v)) on the ACT engine (Copy
activation with scale 0.5, int8 output).  0.5*v is exact (power of two),
and round-to-nearest-even crosses between 0 and +-1 exactly at
0.5*v = +-0.5, i.e. at v = +-1.  Hence
    e == 0       <=>  |v| <= 1   (ties at exactly +-1.0 round to 0)
    clip(e,-1,1) ==  spike (up to the measure-zero v == +-1.0 exact case)
The spike train ships as int8 e and is decoded on host with one clip;
the reset uses (e == 0) * v on DVE (mixed int8/f32 STT).

DRAM layout is [T, C, BS*HW] for both tensors (host transposes), so every
DMA row is 4-16KB contiguous per partition.

Engine budget per core (4.19M elems, DVE 0.96GHz x128 lanes):
    DVE: v = (W mult 0.4) add x    [STT 1x]  7 steps (t=0: v = x)
         W = (e is_equal 0) mult v [STT 1x]  7 steps (skip at t=T-1)
         -> 28 x [C,2048] STT ~= 64us  (the stream bottleneck)
    ACT: e = Copy(0.5*v) -> int8   8 steps ~= 31us
    DMA: 16.78MB in + 4.19MB out ~= 54us across 16 engines
Sharding: data-parallel over batch (axis 1), B=32 -> 4 per core.
"""

import numpy as np

import concourse.bass as bass
import concourse.tile as tile
from concourse import mybir
from concourse.alu_op_type import AluOpType
from concourse.bass_utils import run_bass_kernel_spmd


def _split_sync_waits(nc):
    """This walrus build enforces the ISA limit of one sync wait per
    instruction (two for EventSemaphore), but Tile's sem-assigner freely
    attaches several. Hoist excess waits onto NoOps inserted just before the
    offending instruction on the same engine (waits are monotonic sem-ge, so
    order among them is irrelevant)."""
    ctr = 0
    for f in nc.m.functions:
        for bb in f.blocks:
            il = bb.instructions
            i = 0
            while i < len(il):
                inst = il[i]
                si = getattr(inst, "sync_info", None)
                if si is not None:
                    lim = 2 if isinstance(inst, mybir.InstEventSemaphore) else 1
                    waits = list(si.on_wait)
                    if len(waits) > lim:
                        inst.sync_info = mybir.SyncInfo(
                            on_wait=waits[:lim], on_update=list(si.on_update))
                        for w in waits[lim:]:
                            ctr += 1
                            nop = mybir.InstNoOp(
                                name=f"I-wsplit-{ctr}",
                                engine=inst.engine,
                                bass_nofuse=True,
                                sync_info=mybir.SyncInfo(
                                    on_wait=[w], on_update=[]),
                            )
                            nc.register_instruction(nop, overwrite=True)
                            il.insert(i, nop)
                            i += 1
                i += 1
    return ctr


# ---------------------------------------------------------------------------
# Problem shape (hardcoded per spec: x [T, B, C, H, W] = [8, 32, 128, 32, 32])
T, B, C, H, W = 8, 32, 128, 32, 32
HW = H * W                      # 1024
N_CORES = 8
BS = B // N_CORES               # 4 batches per core
DECAY = float(1.0 - 1.0 / np.float32(5.0 / 3.0))  # 0.4

BPC = 2                         # batches per chunk (chain)
CHUNKS = BS // BPC              # independent chains per core
FREE = BPC * HW                 # free-dim elements per tile
STEPF = BS * HW                 # free-dim elements per full step

F32 = mybir.dt.float32
I8 = mybir.dt.int8
COPY = mybir.ActivationFunctionType.Copy

_NC_CACHE = {}


def _build():
    if "nc" in _NC_CACHE:
        return _NC_CACHE["nc"]
    nc = bass.Bass()
    x = nc.declare_dram_parameter("x", [T, C, STEPF], F32, isOutput=False)
    out_e = nc.declare_dram_parameter("out_e", [T, C, STEPF], I8,
                                      isOutput=True)

    with tile.TileContext(nc) as tc:
        with (
            tc.tile_pool(name="xp", bufs=3) as xp,
            tc.tile_pool(name="vp", bufs=2) as vp,
            tc.tile_pool(name="ep", bufs=3) as ep,
            tc.tile_pool(name="wp", bufs=2) as wp,
        ):
            # preload the ACT Copy table so the first real e-quantize
            # doesn't pay the ~1.3us table load on the critical path
            warm = ep.tile([C, 1], I8, tag="warm")
            warmf = ep.tile([C, 1], F32, tag="warmf")
            nc.vector.memset(warmf[:], 0.0)
            nc.scalar.activation(warm[:], warmf[:], COPY, scale=0.5)

            state = [None] * CHUNKS
            for t in range(T):
                # batched x load; at t=0 split per chunk so compute can
                # start after the first quarter arrives
                xt = xp.tile([C, STEPF], F32, tag="x", name=f"x_{t}")
                if t == 0:
                    # per-chunk slices so the first chunk's compute starts ASAP
                    for s0 in range(CHUNKS):
                        sl0 = slice(s0 * FREE, (s0 + 1) * FREE)
                        nc.sync.dma_start(out=xt[:, sl0], in_=x[t, :, sl0])
                else:
                    nc.sync.dma_start(out=xt[:], in_=x[t])
                et = ep.tile([C, STEPF], I8, tag="e", name=f"e_{t}")
                if t == 0:
                    # ragged slices (smallest first) so the first compute
                    # starts as soon as 0.25MB lands; W0 slices write into
                    # the chunk-shaped state tiles
                    wts = [wp.tile([C, FREE], F32, tag="w", name=f"w_0_{c}")
                           for c in range(CHUNKS)]
                    widths = [HW, HW, HW, HW]
                    off = 0
                    for s, wd in enumerate(widths):
                        sl = slice(off, off + wd)
                        v = xt[:, sl]       # V == 0: v = x_0
                        nc.scalar.activation(et[:, sl], v, COPY, scale=0.5)
                        cch = off // FREE
                        wsl = slice(off - cch * FREE, off - cch * FREE + wd)
                        nc.vector.scalar_tensor_tensor(
                            wts[cch][:, wsl], et[:, sl], 0.0, v,
                            AluOpType.is_equal, AluOpType.mult)
                        nc.scalar.dma_start(out=out_e[t, :, sl],
                                            in_=et[:, sl])
                        off += wd
                    state = wts
                    continue
                for cch in range(CHUNKS):
                    sl = slice(cch * FREE, (cch + 1) * FREE)
                    vt = vp.tile([C, FREE], F32, tag="v", name=f"v_{t}_{cch}")
                    if t == 1:
                        # half-slices: start as soon as W0's first slice lands
                        for h in range(2):
                            hs = slice(h * HW, h * HW + HW)
                            xs = slice(cch * FREE + h * HW,
                                       cch * FREE + h * HW + HW)
                            nc.vector.scalar_tensor_tensor(
                                vt[:, hs], state[cch][:, hs], DECAY,
                                xt[:, xs], AluOpType.mult, AluOpType.add)
                    else:
                        nc.vector.scalar_tensor_tensor(
                            vt[:], state[cch][:], DECAY, xt[:, sl],
                            AluOpType.mult, AluOpType.add)
                    v = vt[:]
                    if t < T - 1:
                        # spike quantize on ACT: e = int8(RNE(0.5*v))
                        nc.scalar.activation(et[:, sl], v, COPY, scale=0.5)
                        w_new = wp.tile([C, FREE], F32, tag="w",
                                        name=f"w_{t}_{cch}")
                        nc.vector.scalar_tensor_tensor(
                            w_new[:], et[:, sl], 0.0, v,
                            AluOpType.is_equal, AluOpType.mult)
                        state[cch] = w_new
                    else:
                        # last step: no state update; quantize on DVE (same
                        # RNE int8 convert) so the tail skips the ACT trip
                        nc.vector.tensor_scalar(
                            et[:, sl], v, 0.5, None, AluOpType.mult)
                    # store from the ACT queue: no cross-engine sem, and it
                    # can't head-of-line-block the x loads on SP
                    nc.scalar.dma_start(out=out_e[t, :, sl], in_=et[:, sl])
    _split_sync_waits(nc)
    _NC_CACHE["nc"] = nc
    return nc


# ---------------------------------------------------------------------------
# Host entry point


def kernel(x: np.ndarray, **run_kwargs) -> np.ndarray:
    assert x.shape == (T, B, C, H, W) and x.dtype == np.float32
    nc = _build()
    xr = np.ascontiguousarray(x).reshape(T, B, C, HW)
    in_maps = []
    for m in range(N_CORES):
        # [T, BS, C, HW] -> [T, C, BS*HW] so DMA rows are contiguous
        xc = xr[:, m * BS:(m + 1) * BS].transpose(0, 2, 1, 3)
        in_maps.append({"x": np.ascontiguousarray(xc).reshape(T, C, STEPF)})
    res = run_bass_kernel_spmd(nc, in_maps, list(range(N_CORES)), **run_kwargs)
    full = np.empty((T, B, C, HW), np.float32)
    for m in range(N_CORES):
        e = np.asarray(res.results[m]["out_e"]).reshape(T, C, BS, HW)
        # decode: spike = clip(e, -1, 1); [T, C, BS, HW] -> [T, BS, C, HW]
        full[:, m * BS:(m + 1) * BS] = np.clip(
            e, -1, 1).astype(np.float32).transpose(0, 2, 1, 3)
    if run_kwargs:
        kernel.last_results = res
    return full.reshape(T, B, C, H, W)


# revision 19
# speedup vs baseline: 1.0197x; 1.0197x over previous
"""LIF neuron (leaky integrate, bidirectional threshold fire, hard reset)
on 8 Trainium2 NeuronCores.

Math (per element, recurrence over T):
    v      = 0.4*V + x_t
    out_t  = (v >= 1) - (v <= -1)               in {-1, 0, +1}
    V'     = (1 - |out_t|) * v                  (hard reset to 0)

Device encoding trick: e = int8(RNE(0.5*v)) on the ACT engine (Copy
activation with scale 0.5, int8 output).  0.5*v is exact (power of two),
and round-to-nearest-even crosses between 0 and +-1 exactly at
0.5*v = +-0.5, i.e. at v = +-1.  Hence
    e == 0       <=>  |v| <= 1   (ties at exactly +-1.0 round to 0)
    clip(e,-1,1) ==  spike (up to the measure-zero v == +-1.0 exact case)
The spike train ships as int8 e and is decoded on host with one clip;
the reset uses (e == 0) * v on DVE (mixed int8/f32 STT).

DRAM layout is [T, C, BS*HW] for both tensors (host transposes), so every
DMA row is 4-16KB contiguous per partition.

Engine budget per core (4.19M elems, DVE 0.96GHz x128 lanes):
    DVE: v = (W mult 0.4) add x    [STT 1x]  7 steps (t=0: v = x)
         W = (e is_equal 0) mult v [STT 1x]  7 steps (skip at t=T-1)
         -> 28 x [C,2048] STT ~= 64us  (the stream bottleneck)
    ACT: e = Copy(0.5*v) -> int8   8 steps ~= 31us
    DMA: 16.78MB in + 4.19MB out ~= 54us across 16 engines
Sharding: data-parallel over batch (axis 1), B=32 -> 4 per core.
"""

import numpy as np

import concourse.bass as bass
import concourse.tile as tile
from concourse import mybir
from concourse.alu_op_type import AluOpType
from concourse.bass_utils import run_bass_kernel_spmd


def _split_sync_waits(nc):
    """This walrus build enforces the ISA limit of one sync wait per
    instruction (two for EventSemaphore), but Tile's sem-assigner freely
    attaches several. Hoist excess waits onto NoOps inserted just before the
    offending instruction on the same engine (waits are monotonic sem-ge, so
    order among them is irrelevant)."""
    ctr = 0
    for f in nc.m.functions:
        for bb in f.blocks:
            il = bb.instructions
            i = 0
            while i < len(il):
                inst = il[i]
                si = getattr(inst, "sync_info", None)
                if si is not None:
                    lim = 2 if isinstance(inst, mybir.InstEventSemaphore) else 1
                    waits = list(si.on_wait)
                    if len(waits) > lim:
                        inst.sync_info = mybir.SyncInfo(
                            on_wait=waits[:lim], on_update=list(si.on_update))
                        for w in waits[lim:]:
                            ctr += 1
                            nop = mybir.InstNoOp(
                                name=f"I-wsplit-{ctr}",
                                engine=inst.engine,
                                bass_nofuse=True,
                                sync_info=mybir.SyncInfo(
                                    on_wait=[w], on_update=[]),
                            )
                            nc.register_instruction(nop, overwrite=True)
                            il.insert(i, nop)
                            i += 1
                i += 1
    return ctr


# ---------------------------------------------------------------------------
# Problem shape (hardcoded per spec: x [T, B, C, H, W] = [8, 32, 128, 32, 32])
T, B, C, H, W = 8, 32, 128, 32, 32
HW = H * W                      # 1024
N_CORES = 8
BS = B // N_CORES               # 4 batches per core
DECAY = float(1.0 - 1.0 / np.float32(5.0 / 3.0))  # 0.4

BPC = 2                         # batches per chunk (chain)
CHUNKS = BS // BPC              # independent chains per core
FREE = BPC * HW                 # free-dim elements per tile
STEPF = BS * HW                 # free-dim elements per full step

F32 = mybir.dt.float32
I8 = mybir.dt.int8
COPY = mybir.ActivationFunctionType.Copy

_NC_CACHE = {}


def _build():
    if "nc" in _NC_CACHE:
        return _NC_CACHE["nc"]
    nc = bass.Bass()
    x = nc.declare_dram_parameter("x", [T, C, STEPF], F32, isOutput=False)
    out_e = nc.declare_dram_parameter("out_e", [T, C, STEPF], I8,
                                      isOutput=True)

    with tile.TileContext(nc) as tc:
        with (
            tc.tile_pool(name="xp", bufs=3) as xp,
            tc.tile_pool(name="vp", bufs=2) as vp,
            tc.tile_pool(name="ep", bufs=3) as ep,
            tc.tile_pool(name="wp", bufs=2) as wp,
        ):
            # preload the ACT Copy table so the first real e-quantize
            # doesn't pay the ~1.3us table load on the critical path
            warm = ep.tile([C, 1], I8, tag="warm")
            warmf = ep.tile([C, 1], F32, tag="warmf")
            nc.vector.memset(warmf[:], 0.0)
            nc.scalar.activation(warm[:], warmf[:], COPY, scale=0.5)

            state = [None] * CHUNKS
            for t in range(T):
                # batched x load; at t=0 split per chunk so compute can
                # start after the first quarter arrives
                xt = xp.tile([C, STEPF], F32, tag="x", name=f"x_{t}")
                if t == 0:
                    # per-chunk slices so the first chunk's compute starts ASAP
                    for s0 in range(CHUNKS):
                        sl0 = slice(s0 * FREE, (s0 + 1) * FREE)
                        nc.sync.dma_start(out=xt[:, sl0], in_=x[t, :, sl0])
                else:
                    nc.sync.dma_start(out=xt[:], in_=x[t])
                et = ep.tile([C, STEPF], I8, tag="e", name=f"e_{t}")
                if t == 0:
                    # ragged slices (smallest first) so the first compute
                    # starts as soon as 0.25MB lands; W0 slices write into
                    # the chunk-shaped state tiles
                    wts = [wp.tile([C, FREE], F32, tag="w", name=f"w_0_{c}")
                           for c in range(CHUNKS)]
                    widths = [HW, HW, HW, HW]
                    off = 0
                    for s, wd in enumerate(widths):
                        sl = slice(off, off + wd)
                        v = xt[:, sl]       # V == 0: v = x_0
                        nc.scalar.activation(et[:, sl], v, COPY, scale=0.5)
                        cch = off // FREE
                        wsl = slice(off - cch * FREE, off - cch * FREE + wd)
                        nc.vector.scalar_tensor_tensor(
                            wts[cch][:, wsl], et[:, sl], 0.0, v,
                            AluOpType.is_equal, AluOpType.mult)
                        nc.scalar.dma_start(out=out_e[t, :, sl],
                                            in_=et[:, sl])
                        off += wd
                    state = wts
                    continue
                for cch in range(CHUNKS):
                    sl = slice(cch * FREE, (cch + 1) * FREE)
                    vt = vp.tile([C, FREE], F32, tag="v", name=f"v_{t}_{cch}")
                    nc.vector.scalar_tensor_tensor(
                        vt[:], state[cch][:], DECAY, xt[:, sl],
                        AluOpType.mult, AluOpType.add)
                    v = vt[:]
                    if t < T - 1:
                        # spike quantize on ACT: e = int8(RNE(0.5*v))
                        nc.scalar.activation(et[:, sl], v, COPY, scale=0.5)
                        w_new = wp.tile([C, FREE], F32, tag="w",
                                        name=f"w_{t}_{cch}")
                        nc.vector.scalar_tensor_tensor(
                            w_new[:], et[:, sl], 0.0, v,
                            AluOpType.is_equal, AluOpType.mult)
                        state[cch] = w_new
                    else:
                        # last step: no state update; quantize on DVE (same
                        # RNE int8 convert) so the tail skips the ACT trip
                        nc.vector.tensor_scalar(
                            et[:, sl], v, 0.5, None, AluOpType.mult)
                    # store from the ACT queue: no cross-engine sem, and it
                    # can't head-of-line-block the x loads on SP
                    nc.scalar.dma_start(out=out_e[t, :, sl], in_=et[:, sl])
    _split_sync_waits(nc)
    _NC_CACHE["nc"] = nc
    return nc


# ---------------------------------------------------------------------------
# Host entry point


def kernel(x: np.ndarray, **run_kwargs) -> np.ndarray:
    assert x.shape == (T, B, C, H, W) and x.dtype == np.float32
    nc = _build()
    xr = np.ascontiguousarray(x).reshape(T, B, C, HW)
    in_maps = []
    for m in range(N_CORES):
        # [T, BS, C, HW] -> [T, C, BS*HW] so DMA rows are contiguous
        xc = xr[:, m * BS:(m + 1) * BS].transpose(0, 2, 1, 3)
        in_maps.append({"x": np.ascontiguousarray(xc).reshape(T, C, STEPF)})
    res = run_bass_kernel_spmd(nc, in_maps, list(range(N_CORES)), **run_kwargs)
    full = np.empty((T, B, C, HW), np.float32)
    for m in range(N_CORES):
        e = np.asarray(res.results[m]["out_e"]).reshape(T, C, BS, HW)
        # decode: spike = clip(e, -1, 1); [T, C, BS, HW] -> [T, BS, C, HW]
        full[:, m * BS:(m + 1) * BS] = np.clip(
            e, -1, 1).astype(np.float32).transpose(0, 2, 1, 3)
    if run_kwargs:
        kernel.last_results = res
    return full.reshape(T, B, C, H, W)
